# revision 10
# baseline (speedup 1.0000x reference)
"""Cosine-similarity kernel (x[16384,512] vs weights[4096,512] -> [16384,4096])
on 8 Trainium2 NeuronCores, data-parallel over the x batch dim.

Per core: x shard [2048,512] fp32, full weights [4096,512] fp32.
  out = normalize(x) @ normalize(w).T

v4 design:
- normalize x/w on DVE+ACT (sum-sq via one DVE tensor_tensor_reduce, then
  reciprocal + sqrt(256*inv) so each normalized side carries a x16 scale),
  emit bf16.
- head-critical tiles (all of x, w blocks 0-1) are transposed on the PE
  (bf16, 1 cyc/row) with a single 3D-AP copy PSUM->SBUF per source tile;
  slack-rich w blocks 2-7 round-trip through a DRAM scratch and come back
  K-major via dma_start_transpose (DMA XBAR), costing the PE nothing.
- main GEMM in bf16 (1 cyc/row), K=512 accumulated over 4 matmuls into
  [128,1024] PSUM groups (2 m-tiles), 3-deep rotation.
- PSUM eviction: dtype-converting copies to int8 (values are 256*cos).
  Early n-blocks evict on Pool only (its queue has no prep work, so
  evictions are never trapped behind normalization); later n-blocks spread
  across ACT/DVE/Pool. int8 output = 4x less writeback; host divides by 256.
- warmup matmuls on zeros are emitted last so the Tile scheduler slots them
  into PE gaps, keeping the p-state ramp warm through the pipeline head.
"""
import numpy as np

B, D, N = 16384, 512, 4096
NCORES = 8
BS = B // NCORES          # 2048 rows per core
MT = BS // 128            # 16 x tiles
NT = N // 128             # 32 w row-tiles
KC = D // 128             # 4 k-chunks
NB = N // 512             # 8 n-blocks of 512
WB = NT // 4              # 8 w blocks of 4 tiles
GPN = 8                   # m-groups per n-block (2 m-tiles each)

W_CNT = 24                # warmup matmuls (gap filler, emitted last)
W_PE_BLOCKS = 2           # w blocks 0..this-1 transposed on PE, rest via XBAR
POOL_ONLY_NB = 3          # n-blocks 0..this-1 evict on Pool only
EV_PATTERN = ["act", "dve", "act", "pool"]
OUT_SCALE = 256.0

_cached = {}


def _build():
    import concourse.bass as bass
    import concourse.mybir as mybir
    import concourse.tile as tile
    from concourse import bacc
    from concourse.masks import make_identity

    F32 = mybir.dt.float32
    BF16 = mybir.dt.bfloat16
    I8 = mybir.dt.int8
    AF = mybir.ActivationFunctionType
    ALU = mybir.AluOpType

    nc = bacc.Bacc(None, target_bir_lowering=False)
    x3 = nc.dram_tensor("x", [MT, 128, D], F32, kind="ExternalInput")
    w3 = nc.dram_tensor("weights", [NT, 128, D], F32, kind="ExternalInput")
    o3 = nc.dram_tensor("out", [MT, 128, N], I8, kind="ExternalOutput")
    ws = nc.dram_tensor("wn_scratch", [NT, 128, D], BF16, kind="Internal")

    with tile.TileContext(nc) as tc:
        with (
            tc.tile_pool(name="const", bufs=1) as const,
            tc.tile_pool(name="fin", bufs=6) as fin,
            tc.tile_pool(name="win", bufs=6) as win,
            tc.tile_pool(name="nrm", bufs=6) as nrm,
            tc.tile_pool(name="nrb", bufs=3) as nrb,
            tc.tile_pool(name="sml", bufs=24) as sml,
            tc.tile_pool(name="scr", bufs=1) as scr,
            tc.tile_pool(name="big", bufs=1) as big,
            tc.tile_pool(name="ob", bufs=24) as ob,
            tc.tile_pool(name="mmps", bufs=3, space="PSUM") as mmps,
            tc.tile_pool(name="trps", bufs=1, space="PSUM") as trps,
            tc.tile_pool(name="wmps", bufs=1, space="PSUM") as wmps,
        ):
            ident = const.tile([128, 128], BF16, name="ident")
            make_identity(nc, ident[:])
            zt = const.tile([128, 512], BF16, name="zt")
            nc.vector.memzero(zt[:])
            sc = scr.tile([128, 512], F32, name="sc")
            # K-major operands: chunk k lives at column offset k*BS / k*N
            xT = big.tile([128, KC * BS], BF16, name="xT")
            wT = big.tile([128, KC * N], BF16, name="wT")

            def norms(src):
                """src [128,512] f32 -> [128,1] f32 tile holding 16/||row||."""
                ss = sml.tile([128, 1], F32, name="ss", tag="ss")
                nc.vector.tensor_tensor_reduce(
                    sc[:], src, src, scale=1.0, scalar=0.0,
                    op0=ALU.mult, op1=ALU.add, accum_out=ss[:])
                inv = sml.tile([128, 1], F32, name="inv", tag="inv")
                nc.vector.reciprocal(inv[:], ss[:])
                rw = sml.tile([128, 1], F32, name="rw", tag="rw")
                nc.scalar.activation(rw[:], inv[:], AF.Sqrt, scale=OUT_SCALE)
                return rw

            def pe_transpose(nb16, dst, col, copy_eng):
                """nb16 [128,512] bf16 -> dst[:, k*stride + col] for 4 chunks."""
                stride = dst.shape[1] // KC
                pt = trps.tile([128, 512], BF16, name="pt", tag="pt")
                for k in range(KC):
                    nc.tensor.transpose(
                        pt[:, k * 128:(k + 1) * 128],
                        nb16[:, k * 128:(k + 1) * 128], ident[:])
                src = pt[:].rearrange("p (k c) -> p k c", k=KC)
                dstap = dst[:].rearrange("p (k n) -> p k n", k=KC)
                dstap = dstap[:, :, col:col + 128]
                if copy_eng == "act":
                    nc.scalar.copy(dstap, src)
                else:
                    nc.vector.tensor_copy(dstap, src)

            def prep_tile_pe(src, dst, col, copy_eng="dve"):
                """normalize one [128,512] f32 tile and PE-transpose into dst."""
                rw = norms(src)
                nb16 = nrm.tile([128, 512], BF16, name="nb16", tag="nb16")
                nc.scalar.mul(nb16[:], src, rw[:])
                pe_transpose(nb16[:], dst, col, copy_eng)

            def w_block_in(b):
                wi = win.tile([128, 2048], F32, name="wi", tag="wi")
                nc.sync.dma_start(
                    wi[:].rearrange("p (j c) -> p j c", j=4),
                    w3[b * 4:(b + 1) * 4, :, :].rearrange("j p c -> p j c"))
                return wi

            def w_block_xbar(b, wi):
                """w block b: normalize, DRAM round-trip, XBAR back."""
                nrmb = nrb.tile([128, 2048], BF16, name="nrmb", tag="nrmb")
                for jj in range(4):
                    rw = norms(wi[:, jj * 512:(jj + 1) * 512])
                    nc.scalar.mul(nrmb[:, jj * 512:(jj + 1) * 512],
                                  wi[:, jj * 512:(jj + 1) * 512], rw[:])
                nc.sync.dma_start(
                    ws[b * 4:(b + 1) * 4, :, :].rearrange("j p c -> p j c"),
                    nrmb[:].rearrange("p (j c) -> p j c", j=4))
                for k in range(KC):
                    src = ws[b * 4:(b + 1) * 4, :, k * 128:(k + 1) * 128]
                    nc.sync.dma_start_transpose(
                        wT[:, k * N + b * 512:k * N + (b + 1) * 512],
                        src.rearrange("j p c -> (j p) c"))

            def w_block_pe(b):
                """w block b via per-tile loads + PE transposes."""
                for jj in range(4):
                    j = b * 4 + jj
                    ft = fin.tile([128, 512], F32, name="wf", tag="ff")
                    nc.sync.dma_start(ft[:], w3[j, :, :])
                    prep_tile_pe(ft[:], wT, j * 128, copy_eng="act")

            def x_tile_in(m):
                ft = fin.tile([128, 512], F32, name="xf", tag="ff")
                nc.sync.dma_start(ft[:], x3[m, :, :])
                return ft

            def x_tile_prep(ft, m):
                prep_tile_pe(ft[:], xT, m * 128, copy_eng="dve")

            def main_group(nb, g, ev):
                """2 m-tiles x 512 n cosine block + eviction + writeback."""
                pm = mmps.tile([128, 1024], F32, name="pm", tag="pm")
                for mi in range(2):
                    m = g * 2 + mi
                    for k in range(KC):
                        nc.tensor.matmul(
                            pm[:, mi * 512:(mi + 1) * 512],
                            xT[:, k * BS + m * 128:k * BS + (m + 1) * 128],
                            wT[:, k * N + nb * 512:k * N + (nb + 1) * 512],
                            start=(k == 0), stop=(k == KC - 1))
                obt = ob.tile([128, 1024], I8, name="obt", tag="obt")
                if ev == "act":
                    nc.scalar.copy(obt[:], pm[:])
                elif ev == "dve":
                    nc.vector.tensor_copy(obt[:], pm[:])
                else:
                    nc.gpsimd.tensor_copy(obt[:], pm[:])
                out_ap = o3[g * 2:(g + 1) * 2, :, nb * 512:(nb + 1) * 512]
                nc.sync.dma_start(out_ap.rearrange("m p c -> p m c"),
                                  obt[:].rearrange("p (m c) -> p m c", m=2))

            # ---- emission ----
            # head: w0 tiles + first x tiles, interleaved by consumption order
            w_block_pe(0)
            xf = {m: x_tile_in(m) for m in range(6)}
            for m in range(4):
                x_tile_prep(xf[m], m)
            w_block_pe(1)
            for m in range(4, 6):
                x_tile_prep(xf[m], m)
            for m in range(6, 10):
                f = x_tile_in(m)
                x_tile_prep(f, m)
            wi2 = w_block_in(2)
            for m in range(10, MT):
                f = x_tile_in(m)
                x_tile_prep(f, m)
            wis = {b: w_block_in(b) for b in range(3, WB)}
            w_block_xbar(2, wi2)
            for b in range(3, WB):
                w_block_xbar(b, wis[b])

            def ev_for(nb, g):
                if nb < POOL_ONLY_NB:
                    return "pool"
                return EV_PATTERN[g % len(EV_PATTERN)]

            # interleave nb0 tail with nb1 head to ride out x arrival
            order = [(0, 0), (0, 1), (0, 2), (1, 0), (0, 3), (1, 1),
                     (0, 4), (1, 2), (0, 5), (1, 3), (0, 6), (0, 7),
                     (1, 4), (1, 5), (1, 6), (1, 7)]
            for nb, g in order:
                main_group(nb, g, ev_for(nb, g))
            for nb in range(2, NB):
                for g in range(GPN):
                    main_group(nb, g, ev_for(nb, g))

            # warmups last: scheduler treats them as always-ready gap filler
            pmW = wmps.tile([128, 512], F32, name="pmW")
            for _ in range(W_CNT):
                nc.tensor.matmul(pmW[:], zt[:, 0:128], zt[:],
                                 start=True, stop=True, skip_group_check=True)
    nc.compile()
    return nc


def kernel(x: np.ndarray, weights: np.ndarray) -> np.ndarray:
    from concourse.bass_utils import run_bass_kernel_spmd

    if "nc" not in _cached:
        _cached["nc"] = _build()
    nc = _cached["nc"]

    x = np.ascontiguousarray(x, dtype=np.float32)
    weights = np.ascontiguousarray(weights, dtype=np.float32)
    w3 = weights.reshape(NT, 128, D)
    in_maps = [
        {"x": x[i * BS:(i + 1) * BS].reshape(MT, 128, D), "weights": w3}
        for i in range(NCORES)
    ]
    res = run_bass_kernel_spmd(nc, in_maps, list(range(NCORES)))
    outs = [
        res.results[i]["out"].astype(np.float32).reshape(BS, N) / OUT_SCALE
        for i in range(NCORES)
    ]
    return np.concatenate(outs, axis=0)


# revision 12
# speedup vs baseline: 1.0817x; 1.0817x over previous
"""Cosine-similarity kernel (x[16384,512] vs weights[4096,512] -> [16384,4096])
on 8 Trainium2 NeuronCores, data-parallel over the x batch dim.

Per core: x shard [2048,512] fp32, full weights [4096,512] fp32.
  out = normalize(x) @ normalize(w).T

v4 design:
- normalize x/w on DVE+ACT (sum-sq via one DVE tensor_tensor_reduce, then
  reciprocal + sqrt(256*inv) so each normalized side carries a x16 scale),
  emit bf16.
- head-critical tiles (all of x, w blocks 0-1) are transposed on the PE
  (bf16, 1 cyc/row) with a single 3D-AP copy PSUM->SBUF per source tile;
  slack-rich w blocks 2-7 round-trip through a DRAM scratch and come back
  K-major via dma_start_transpose (DMA XBAR), costing the PE nothing.
- main GEMM in bf16 (1 cyc/row), K=512 accumulated over 4 matmuls into
  [128,1024] PSUM groups (2 m-tiles), 3-deep rotation.
- PSUM eviction: dtype-converting copies to int8 (values are 256*cos).
  Early n-blocks evict on Pool only (its queue has no prep work, so
  evictions are never trapped behind normalization); later n-blocks spread
  across ACT/DVE/Pool. int8 output = 4x less writeback; host divides by 256.
- warmup matmuls on zeros are emitted last so the Tile scheduler slots them
  into PE gaps, keeping the p-state ramp warm through the pipeline head.
"""
import numpy as np

B, D, N = 16384, 512, 4096
NCORES = 8
BS = B // NCORES          # 2048 rows per core
MT = BS // 128            # 16 x tiles
NT = N // 128             # 32 w row-tiles
KC = D // 128             # 4 k-chunks
NB = N // 512             # 8 n-blocks of 512
WB = NT // 4              # 8 w blocks of 4 tiles
GPN = 8                   # m-groups per n-block (2 m-tiles each)

W_CNT = 24                # warmup matmuls (gap filler, emitted last)
W_PE_BLOCKS = 2           # w blocks 0..this-1 transposed on PE, rest via XBAR
POOL_ONLY_NB = 3          # n-blocks 0..this-1 evict on Pool only
EV_PATTERN = ["pool", "act", "dve", "pool"]
OUT_SCALE = 256.0

_cached = {}


def _build():
    import concourse.bass as bass
    import concourse.mybir as mybir
    import concourse.tile as tile
    from concourse import bacc
    from concourse.masks import make_identity

    F32 = mybir.dt.float32
    BF16 = mybir.dt.bfloat16
    I8 = mybir.dt.int8
    AF = mybir.ActivationFunctionType
    ALU = mybir.AluOpType

    nc = bacc.Bacc(None, target_bir_lowering=False)
    x3 = nc.dram_tensor("x", [MT, 128, D], F32, kind="ExternalInput")
    w3 = nc.dram_tensor("weights", [NT, 128, D], F32, kind="ExternalInput")
    o3 = nc.dram_tensor("out", [MT, 128, N], I8, kind="ExternalOutput")
    ws = nc.dram_tensor("wn_scratch", [NT, 128, D], BF16, kind="Internal")

    with tile.TileContext(nc) as tc:
        with (
            tc.tile_pool(name="const", bufs=1) as const,
            tc.tile_pool(name="fin", bufs=6) as fin,
            tc.tile_pool(name="win", bufs=6) as win,
            tc.tile_pool(name="nrm", bufs=6) as nrm,
            tc.tile_pool(name="nrb", bufs=3) as nrb,
            tc.tile_pool(name="sml", bufs=24) as sml,
            tc.tile_pool(name="scr", bufs=1) as scr,
            tc.tile_pool(name="big", bufs=1) as big,
            tc.tile_pool(name="ob", bufs=24) as ob,
            tc.tile_pool(name="mmps", bufs=3, space="PSUM") as mmps,
            tc.tile_pool(name="trps", bufs=1, space="PSUM") as trps,
            tc.tile_pool(name="wmps", bufs=1, space="PSUM") as wmps,
        ):
            ident = const.tile([128, 128], BF16, name="ident")
            make_identity(nc, ident[:])
            zt = const.tile([128, 512], BF16, name="zt")
            nc.vector.memzero(zt[:])
            sc = scr.tile([128, 512], F32, name="sc")
            # K-major operands: chunk k lives at column offset k*BS / k*N
            xT = big.tile([128, KC * BS], BF16, name="xT")
            wT = big.tile([128, KC * N], BF16, name="wT")

            def norms(src):
                """src [128,512] f32 -> [128,1] f32 tile holding 16/||row||."""
                ss = sml.tile([128, 1], F32, name="ss", tag="ss")
                nc.vector.tensor_tensor_reduce(
                    sc[:], src, src, scale=1.0, scalar=0.0,
                    op0=ALU.mult, op1=ALU.add, accum_out=ss[:])
                inv = sml.tile([128, 1], F32, name="inv", tag="inv")
                nc.vector.reciprocal(inv[:], ss[:])
                rw = sml.tile([128, 1], F32, name="rw", tag="rw")
                nc.scalar.activation(rw[:], inv[:], AF.Sqrt, scale=OUT_SCALE)
                return rw

            def pe_transpose(nb16, dst, col, copy_eng):
                """nb16 [128,512] bf16 -> dst[:, k*stride + col] for 4 chunks."""
                stride = dst.shape[1] // KC
                pt = trps.tile([128, 512], BF16, name="pt", tag="pt")
                for k in range(KC):
                    nc.tensor.transpose(
                        pt[:, k * 128:(k + 1) * 128],
                        nb16[:, k * 128:(k + 1) * 128], ident[:])
                src = pt[:].rearrange("p (k c) -> p k c", k=KC)
                dstap = dst[:].rearrange("p (k n) -> p k n", k=KC)
                dstap = dstap[:, :, col:col + 128]
                if copy_eng == "act":
                    nc.scalar.copy(dstap, src)
                else:
                    nc.vector.tensor_copy(dstap, src)

            def prep_tile_pe(src, dst, col, copy_eng="dve"):
                """normalize one [128,512] f32 tile and PE-transpose into dst."""
                rw = norms(src)
                nb16 = nrm.tile([128, 512], BF16, name="nb16", tag="nb16")
                nc.scalar.mul(nb16[:], src, rw[:])
                pe_transpose(nb16[:], dst, col, copy_eng)

            def w_block_in(b):
                wi = win.tile([128, 2048], F32, name="wi", tag="wi")
                nc.sync.dma_start(
                    wi[:].rearrange("p (j c) -> p j c", j=4),
                    w3[b * 4:(b + 1) * 4, :, :].rearrange("j p c -> p j c"))
                return wi

            def w_block_xbar(b, wi):
                """w block b: normalize (batched norms), round-trip, XBAR."""
                ss4 = sml.tile([128, 4], F32, name="ss4", tag="ss4")
                for jj in range(4):
                    nc.vector.tensor_tensor_reduce(
                        sc[:], wi[:, jj * 512:(jj + 1) * 512],
                        wi[:, jj * 512:(jj + 1) * 512], scale=1.0, scalar=0.0,
                        op0=ALU.mult, op1=ALU.add,
                        accum_out=ss4[:, jj:jj + 1])
                inv4 = sml.tile([128, 4], F32, name="inv4", tag="inv4")
                nc.vector.reciprocal(inv4[:], ss4[:])
                rw4 = sml.tile([128, 4], F32, name="rw4", tag="rw4")
                nc.scalar.activation(rw4[:], inv4[:], AF.Sqrt, scale=OUT_SCALE)
                nrmb = nrb.tile([128, 2048], BF16, name="nrmb", tag="nrmb")
                for jj in range(4):
                    nc.scalar.mul(nrmb[:, jj * 512:(jj + 1) * 512],
                                  wi[:, jj * 512:(jj + 1) * 512],
                                  rw4[:, jj:jj + 1])
                nc.sync.dma_start(
                    ws[b * 4:(b + 1) * 4, :, :].rearrange("j p c -> p j c"),
                    nrmb[:].rearrange("p (j c) -> p j c", j=4))
                for k in range(KC):
                    src = ws[b * 4:(b + 1) * 4, :, k * 128:(k + 1) * 128]
                    nc.sync.dma_start_transpose(
                        wT[:, k * N + b * 512:k * N + (b + 1) * 512],
                        src.rearrange("j p c -> (j p) c"))

            def w_block_pe(b):
                """w block b via per-tile loads + PE transposes."""
                for jj in range(4):
                    j = b * 4 + jj
                    ft = fin.tile([128, 512], F32, name="wf", tag="ff")
                    nc.sync.dma_start(ft[:], w3[j, :, :])
                    prep_tile_pe(ft[:], wT, j * 128, copy_eng="act")

            def x_tile_in(m):
                ft = fin.tile([128, 512], F32, name="xf", tag="ff")
                nc.sync.dma_start(ft[:], x3[m, :, :])
                return ft

            def x_tile_prep(ft, m):
                prep_tile_pe(ft[:], xT, m * 128, copy_eng="dve")

            def main_group(nb, g, ev):
                """2 m-tiles x 512 n cosine block + eviction + writeback."""
                pm = mmps.tile([128, 1024], F32, name="pm", tag="pm")
                for mi in range(2):
                    m = g * 2 + mi
                    for k in range(KC):
                        nc.tensor.matmul(
                            pm[:, mi * 512:(mi + 1) * 512],
                            xT[:, k * BS + m * 128:k * BS + (m + 1) * 128],
                            wT[:, k * N + nb * 512:k * N + (nb + 1) * 512],
                            start=(k == 0), stop=(k == KC - 1))
                obt = ob.tile([128, 1024], I8, name="obt", tag="obt")
                if ev == "act":
                    nc.scalar.copy(obt[:], pm[:])
                elif ev == "dve":
                    nc.vector.tensor_copy(obt[:], pm[:])
                else:
                    nc.gpsimd.tensor_copy(obt[:], pm[:])
                out_ap = o3[g * 2:(g + 1) * 2, :, nb * 512:(nb + 1) * 512]
                nc.sync.dma_start(out_ap.rearrange("m p c -> p m c"),
                                  obt[:].rearrange("p (m c) -> p m c", m=2))

            # ---- emission ----
            # head: w0 tiles + first x tiles, interleaved by consumption order
            w_block_pe(0)
            xf = {m: x_tile_in(m) for m in range(6)}
            for m in range(4):
                x_tile_prep(xf[m], m)
            w_block_pe(1)
            for m in range(4, 6):
                x_tile_prep(xf[m], m)
            for m in range(6, 10):
                f = x_tile_in(m)
                x_tile_prep(f, m)
            wi2 = w_block_in(2)
            for m in range(10, MT):
                f = x_tile_in(m)
                x_tile_prep(f, m)
            wis = {b: w_block_in(b) for b in range(3, WB)}
            w_block_xbar(2, wi2)
            for b in range(3, WB):
                w_block_xbar(b, wis[b])

            def ev_for(nb, g):
                if nb < POOL_ONLY_NB:
                    return "pool"
                return EV_PATTERN[g % len(EV_PATTERN)]

            # interleave nb0 tail with nb1 head to ride out x arrival
            order = [(0, 0), (0, 1), (0, 2), (1, 0), (0, 3), (1, 1),
                     (0, 4), (1, 2), (0, 5), (1, 3), (0, 6), (0, 7),
                     (1, 4), (1, 5), (1, 6), (1, 7)]
            for nb, g in order:
                main_group(nb, g, ev_for(nb, g))
            for nb in range(2, NB):
                for g in range(GPN):
                    main_group(nb, g, ev_for(nb, g))

            # warmups last: scheduler treats them as always-ready gap filler
            pmW = wmps.tile([128, 512], F32, name="pmW")
            for _ in range(W_CNT):
                nc.tensor.matmul(pmW[:], zt[:, 0:128], zt[:],
                                 start=True, stop=True, skip_group_check=True)
    nc.compile()
    return nc


def kernel(x: np.ndarray, weights: np.ndarray) -> np.ndarray:
    from concourse.bass_utils import run_bass_kernel_spmd

    if "nc" not in _cached:
        _cached["nc"] = _build()
    nc = _cached["nc"]

    x = np.ascontiguousarray(x, dtype=np.float32)
    weights = np.ascontiguousarray(weights, dtype=np.float32)
    w3 = weights.reshape(NT, 128, D)
    in_maps = [
        {"x": x[i * BS:(i + 1) * BS].reshape(MT, 128, D), "weights": w3}
        for i in range(NCORES)
    ]
    res = run_bass_kernel_spmd(nc, in_maps, list(range(NCORES)))
    outs = [
        res.results[i]["out"].astype(np.float32).reshape(BS, N) / OUT_SCALE
        for i in range(NCORES)
    ]
    return np.concatenate(outs, axis=0)
